# revision 22
# baseline (speedup 1.0000x reference)
"""Trainium2 Bass SPMD kernel: 16-head MHA (B=4, S=2048, D=1024), fp32.

Sharding: 8 cores = 4 batches x 2 head-groups (8 heads each). Host pre-
transposes activations to [D, S] and pre-slices/transposes weights, so the
device never transposes anything:

  - Q/K projections produce QT/KT in [d_local, S] layout (head dim on
    partitions) which directly feeds the scores matmul.
  - Scores are computed transposed ([t, s] in PSUM), exp'd on ScalarE
    (scale=1/8 folded in, no max-subtraction: scores*0.125 max ~10, exp
    ~3e4, fine in fp32), written to SBUF as fp32r.
  - V is produced in natural [t, d] layout with an appended ones column, so
    the PV matmul yields both the unnormalized output (rows 0..63) and the
    softmax denominator (row 64) in one pass.
  - Normalization: reciprocal of the denominator row + K=1 ones-matmul to
    broadcast it across partitions + one DVE multiply.
  - O-projection contracts attn^T [d_local, s] tiles against w_o columns;
    per-core partial outputs are summed (+b_o) on the host.

All matmuls run in float32r (full PE rate at N=512, ~1.6e-4 rel err).
"""
import numpy as np

import concourse.bass as bass
import concourse.mybir as mybir
from concourse.tile import TileContext
from concourse.bass_utils import run_bass_kernel_spmd

F32 = mybir.dt.float32
F32R = mybir.dt.float32r
AF = mybir.ActivationFunctionType

B, S, D = 4, 2048, 1024
H, DH = 16, 64
HL = 8        # heads per core
DL = HL * DH  # 512 local model dims
P = 128
SC = 512      # s-chunk width
NCH = S // SC  # 4 s-chunks
ND = D // P    # 8 contraction subtiles for D
NM = DL // P   # 4 m-tiles of local outputs
NT = S // P    # 16 t-tiles

_MAX_WAITS = 1


def _split_excess_waits(nc, max_waits=_MAX_WAITS):
    """walrus here rejects >1 sync-wait per instruction; spill extras onto
    same-engine NoOps inserted before the instruction."""
    f = nc.m.functions[0]
    n = 0
    for bb in f.blocks:
        changed = False
        out = []
        for inst in bb.instructions:
            si = inst.sync_info
            if si is not None and len(si.on_wait) > max_waits:
                waits = list(si.on_wait)
                keep = waits[-max_waits:]
                spill = waits[:-max_waits]
                for i in range(0, len(spill), max_waits):
                    nop = mybir.InstNoOp(name=f"WSPILL-{n}", ins=[], outs=[])
                    n += 1
                    nop.engine = inst.engine
                    nop.sync_info = mybir.SyncInfo(
                        on_wait=spill[i : i + max_waits], on_update=[]
                    )
                    nc.register_instruction(nop, overwrite=True)
                    out.append(nop)
                inst.sync_info = mybir.SyncInfo(
                    on_wait=keep, on_update=list(si.on_update)
                )
                changed = True
            out.append(inst)
        if changed:
            bb.instructions = out
    return n


def build():
    nc = bass.Bass()
    xq = nc.dram_tensor("xq", [D, S], F32R, kind="ExternalInput")
    xk = nc.dram_tensor("xk", [D, S], F32R, kind="ExternalInput")
    xv = nc.dram_tensor("xv", [D, S], F32R, kind="ExternalInput")
    wq = nc.dram_tensor("wq", [D, DL], F32R, kind="ExternalInput")
    wk = nc.dram_tensor("wk", [D, DL], F32R, kind="ExternalInput")
    wv = nc.dram_tensor("wv", [D, DL], F32R, kind="ExternalInput")
    wo = nc.dram_tensor("wo", [DL, D], F32R, kind="ExternalInput")
    bq = nc.dram_tensor("bq", [DL], F32, kind="ExternalInput")
    bk = nc.dram_tensor("bk", [DL], F32, kind="ExternalInput")
    bv = nc.dram_tensor("bv", [DL], F32R, kind="ExternalInput")
    out = nc.dram_tensor("out", [S, D], F32, kind="ExternalOutput")

    xq_r = xq.rearrange("(o p) s -> p o s", p=P)
    xk_r = xk.rearrange("(o p) s -> p o s", p=P)
    xv_r = xv.rearrange("(o p) s -> p o s", p=P)
    wq_r = wq.rearrange("(o p) m -> p o m", p=P)
    wk_r = wk.rearrange("(o p) m -> p o m", p=P)
    wv_r = wv.rearrange("(o p) m -> p o m", p=P)
    wo_r = wo.rearrange("(o p) n -> p o n", p=P)
    out_r = out.rearrange("(so p) n -> p so n", p=P)

    with TileContext(nc) as tc:
        with (
            tc.tile_pool(name="persist", bufs=1) as persist,
            tc.tile_pool(name="wpool", bufs=1) as wpool,
            tc.tile_pool(name="xpool", bufs=3) as xpool,
            tc.tile_pool(name="expp", bufs=2) as expp,
            tc.tile_pool(name="attnp", bufs=2) as attnp,
            tc.tile_pool(name="osb", bufs=2) as osbp,
            tc.tile_pool(name="nrm", bufs=2) as nrm,
            tc.tile_pool(name="ps_big", bufs=1, space="PSUM") as ps_big,
            tc.tile_pool(name="ps_pv", bufs=1, space="PSUM") as ps_pv,
            tc.tile_pool(name="ps_sm", bufs=2, space="PSUM") as ps_sm,
        ):
            qt = persist.tile([P, NM, S], F32R, tag="qt")
            kt = persist.tile([P, NM, S], F32R, tag="kt")
            vaug = persist.tile([P, NT, HL, 66], F32R, tag="vaug")
            wq_t = persist.tile([P, ND, DL], F32R, tag="wq")
            ones_f = persist.tile([P, P], F32, tag="ones_f")
            ones_r = persist.tile([P, P], F32R, tag="ones_r")
            bq_sb = persist.tile([P, NM], F32, tag="bq")
            bk_sb = persist.tile([P, NM], F32, tag="bk")
            bv_t = persist.tile([P, DL], F32R, tag="bv_t")
            bv_bc = persist.tile([P, DL], F32, tag="bv_bc")

            # ---- constants / biases ----
            nc.vector.memset(ones_f[:], 1.0)
            nc.vector.tensor_copy(ones_r[:], ones_f[:])
            nc.sync.dma_start(bq_sb[:], bq.rearrange("(o p) -> p o", p=P))
            nc.sync.dma_start(bk_sb[:], bk.rearrange("(o p) -> p o", p=P))
            nc.sync.dma_start(bv_t[0:1, :], bv[None, :])
            ps = ps_sm.tile([P, SC], F32, tag="sm")
            nc.tensor.matmul(ps[:], ones_r[0:1, 0:P], bv_t[0:1, :], start=True,
                             stop=True)
            nc.vector.tensor_copy(bv_bc[:], ps[:])
            # ones columns of V_aug
            of = ones_f[:, 0:NT * HL].rearrange("p (a b) -> p a b", a=NT)
            nc.vector.tensor_copy(vaug[:, :, :, 0:1], of[:, :, :, None])
            nc.vector.tensor_copy(vaug[:, :, :, 65:66], of[:, :, :, None])

            def proj_chunk(dst, w_tile, x_r, c, bias_sb):
                xa = xpool.tile([P, ND // 2, SC], F32R, tag="x")
                xb = xpool.tile([P, ND // 2, SC], F32R, tag="x")
                nc.sync.dma_start(xa[:], x_r[:, 0 : ND // 2, c * SC : (c + 1) * SC])
                nc.sync.dma_start(xb[:], x_r[:, ND // 2 : ND, c * SC : (c + 1) * SC])
                halves = (xa, xb)
                for m in range(NM):
                    psm = ps_sm.tile([P, SC], F32, tag="sm")
                    for k in range(ND):
                        nc.tensor.matmul(
                            psm[:],
                            w_tile[:, k, m * P : (m + 1) * P],
                            halves[k // 4][:, k % 4, :],
                            start=(k == 0),
                            stop=(k == ND - 1),
                        )
                    nc.vector.tensor_add(
                        dst[:, m, c * SC : (c + 1) * SC],
                        psm[:],
                        bias_sb[:, m : m + 1].to_broadcast((P, SC)),
                    )

            def vproj_chunk(wv_t, c):
                xa = xpool.tile([P, ND // 2, SC], F32R, tag="x")
                xb = xpool.tile([P, ND // 2, SC], F32R, tag="x")
                nc.sync.dma_start(xa[:], xv_r[:, 0 : ND // 2, c * SC : (c + 1) * SC])
                nc.sync.dma_start(xb[:], xv_r[:, ND // 2 : ND, c * SC : (c + 1) * SC])
                halves = (xa, xb)
                for i in range(4):
                    t_o = c * 4 + i
                    psm = ps_sm.tile([P, SC], F32, tag="sm")
                    for k in range(ND):
                        nc.tensor.matmul(
                            psm[:],
                            halves[k // 4][:, k % 4, i * P : (i + 1) * P],
                            wv_t[:, k, :],
                            start=(k == 0),
                            stop=(k == ND - 1),
                        )
                    for h in range(HL):
                        nc.vector.tensor_add(
                            vaug[:, t_o, h, 1:65],
                            psm[:, h * DH : (h + 1) * DH],
                            bv_bc[:, h * DH : (h + 1) * DH],
                        )

            def normalize_pre(pv):
                """Consume the PV psum right away on DVE (frees the psum slot):
                copy unnormalized rows, reciprocal of the denominator row."""
                raw = nrm.tile([P, SC], F32, tag="raw")
                rec = nrm.tile([P, SC], F32R, tag="rec")
                nc.scalar.copy(raw[0:64, :], pv[0:64, :])
                nc.vector.tensor_copy(raw[64:65, :], pv[64:65, :])
                with nc.allow_low_precision(reason="fp32r recip for matmul rhs"):
                    nc.vector.reciprocal(rec[64:65, :], raw[64:65, :])
                return raw, rec

            def normalize_post(raw, rec, dst_lo, dst_hi_dma):
                """Broadcast 1/denom across partitions (K=1 matmul) and apply."""
                bc = ps_sm.tile([P, SC], F32, tag="sm")
                nc.tensor.matmul(bc[0:64, :], ones_r[64:65, 0:64], rec[64:65, :],
                                 start=True, stop=True)
                if dst_hi_dma is None:
                    nc.vector.tensor_mul(dst_lo, bc[0:64, :], raw[0:64, :])
                else:
                    tmp = nrm.tile([P, SC], F32R, tag="tmp")
                    nc.vector.tensor_mul(tmp[0:64, :], bc[0:64, :], raw[0:64, :])
                    nc.sync.dma_start(dst_hi_dma, tmp[0:64, :])

            # ---- projections (prefix) ----
            nc.sync.dma_start(wq_t[:], wq_r[:])
            proj_chunk(qt, wq_t, xq_r, 0, bq_sb)
            wk_t = wpool.tile([P, ND, DL], F32R, tag="w")
            nc.sync.dma_start(wk_t[:], wk_r[:])
            for c in range(NCH):
                proj_chunk(kt, wk_t, xk_r, c, bk_sb)
            wv_t = wpool.tile([P, ND, DL], F32R, tag="w")
            nc.sync.dma_start(wv_t[:], wv_r[:])
            for c in range(NCH):
                vproj_chunk(wv_t, c)
            wo_t = wpool.tile([P, NM, D], F32R, tag="w")
            nc.sync.dma_start(wo_t[:], wo_r[:])

            # ---- attention + o-proj, per s-chunk ----
            def oproj_chunk(c, attn_t):
                for st in range(4):
                    for n in range(2):
                        psm = ps_sm.tile([P, SC], F32, tag="sm")
                        for do in range(NM):
                            nc.tensor.matmul(
                                psm[:],
                                attn_t[:, do, st * P : (st + 1) * P],
                                wo_t[:, do, n * SC : (n + 1) * SC],
                                start=(do == 0),
                                stop=(do == NM - 1),
                            )
                        ob = osbp.tile([P, SC], F32, tag="ob")
                        nc.vector.tensor_copy(ob[:], psm[:])
                        nc.sync.dma_start(
                            out_r[:, c * 4 + st, n * SC : (n + 1) * SC], ob[:]
                        )

            pending_oproj = None
            for c in range(NCH):
                attn_t = attnp.tile([P, NM, SC], F32R, tag="attn")
                cs = slice(c * SC, (c + 1) * SC)
                deferred = None
                for p in range(NM):
                    pv_e = ps_pv.tile([P, SC], F32, tag="pv_e")
                    pv_o = ps_pv.tile([P, SC], F32, tag="pv_o")

                    def emit_pv(ex, ta, tb, p=p, pv_e=pv_e, pv_o=pv_o):
                        for j, (t, hh) in enumerate(
                            [(ta, 2 * p), (tb, 2 * p), (ta, 2 * p + 1),
                             (tb, 2 * p + 1)]
                        ):
                            pv = pv_e if j < 2 else pv_o
                            nc.tensor.matmul(
                                pv[0:65, :],
                                vaug[:, t, hh, 1:66],
                                ex[:, j, :],
                                start=(t == 0),
                                stop=(t == NT - 1),
                            )

                    pend = None
                    for g in range(NT // 2):
                        ta, tb = 2 * g, 2 * g + 1
                        big = ps_big.tile([P, 4, SC], F32, tag="big")
                        for j, (t, lo) in enumerate(
                            [(ta, 0), (tb, 0), (ta, 64), (tb, 64)]
                        ):
                            nc.tensor.matmul(
                                big[:, j, :],
                                kt[lo : lo + 64, p, t * P : (t + 1) * P],
                                qt[lo : lo + 64, p, cs],
                                start=True,
                                stop=True,
                                tile_position=(lo, 0),
                            )
                        ex = expp.tile([P, 4, SC], F32R, tag="ex")
                        nc.scalar.activation(ex[:], big[:], AF.Exp, scale=0.125)
                        if g == 4 and deferred is not None:
                            deferred()
                            deferred = None
                        if g == 5 and p == 0 and pending_oproj is not None:
                            pending_oproj()
                            pending_oproj = None
                        if g == 2 and p == 3 and c < NCH - 1:
                            proj_chunk(qt, wq_t, xq_r, c + 1, bq_sb)
                        if pend is not None:
                            emit_pv(*pend)
                        pend = (ex, ta, tb)
                    emit_pv(*pend)
                    raw_e, rec_e = normalize_pre(pv_e)
                    raw_o, rec_o = normalize_pre(pv_o)

                    def make_deferred(raw_e=raw_e, rec_e=rec_e, raw_o=raw_o,
                                      rec_o=rec_o, p=p, attn_t=attn_t):
                        def run():
                            normalize_post(raw_e, rec_e, attn_t[0:64, p, :], None)
                            normalize_post(raw_o, rec_o, None,
                                           attn_t[64:128, p, :])
                        return run

                    deferred = make_deferred()
                deferred()

                def make_oproj(c=c, attn_t=attn_t):
                    return lambda: oproj_chunk(c, attn_t)

                pending_oproj = make_oproj()
            pending_oproj()

    _split_excess_waits(nc)
    return nc


_CACHE = {}


def _get_nc():
    if "nc" not in _CACHE:
        _CACHE["nc"] = build()
    return _CACHE["nc"]


def _f32(x):
    return np.asarray(x).astype(np.float32, copy=False)


def _prep_core_inputs(c, q, k, v, w_q, b_q, w_k, b_k, w_v, b_v, w_o, b_o):
    b, hg = c // 2, c % 2
    hs = slice(hg * DL, hg * DL + DL)
    return {
        "xq": np.ascontiguousarray(q[b].T),
        "xk": np.ascontiguousarray(k[b].T),
        "xv": np.ascontiguousarray(v[b].T),
        "wq": np.ascontiguousarray(w_q[hs, :].T),
        "wk": np.ascontiguousarray(w_k[hs, :].T),
        "wv": np.ascontiguousarray(w_v[hs, :].T),
        "wo": np.ascontiguousarray(w_o[:, hs].T),
        "bq": np.ascontiguousarray(b_q[hs]),
        "bk": np.ascontiguousarray(b_k[hs]),
        "bv": np.ascontiguousarray(b_v[hs]),
    }


def kernel(q, k, v, w_q, b_q, w_k, b_k, w_v, b_v, w_o, b_o):
    q, k, v = _f32(q), _f32(k), _f32(v)
    w_q, b_q = _f32(w_q), _f32(b_q)
    w_k, b_k = _f32(w_k), _f32(b_k)
    w_v, b_v = _f32(w_v), _f32(b_v)
    w_o, b_o = _f32(w_o), _f32(b_o)

    nc = _get_nc()
    in_maps = [
        _prep_core_inputs(c, q, k, v, w_q, b_q, w_k, b_k, w_v, b_v, w_o, b_o)
        for c in range(8)
    ]
    res = run_bass_kernel_spmd(nc, in_maps, core_ids=list(range(8)))
    out = np.empty((B, S, D), np.float32)
    for b in range(B):
        out[b] = res.results[2 * b]["out"] + res.results[2 * b + 1]["out"] + b_o
    return out


# revision 23
# speedup vs baseline: 1.1581x; 1.1581x over previous
"""Trainium2 Bass SPMD kernel: 16-head MHA (B=4, S=2048, D=1024), fp32.

Sharding: 8 cores = 4 batches x 2 head-groups (8 heads each). Host pre-
transposes activations to [D, S] and pre-slices/transposes weights, so the
device never transposes anything:

  - Q/K projections produce QT/KT in [d_local, S] layout (head dim on
    partitions) which directly feeds the scores matmul.
  - Scores are computed transposed ([t, s] in PSUM), exp'd on ScalarE
    (scale=1/8 folded in, no max-subtraction: scores*0.125 max ~10, exp
    ~3e4, fine in fp32), written to SBUF as fp32r.
  - V is produced in natural [t, d] layout with an appended ones column, so
    the PV matmul yields both the unnormalized output (rows 0..63) and the
    softmax denominator (row 64) in one pass.
  - Normalization: reciprocal of the denominator row + K=1 ones-matmul to
    broadcast it across partitions + one DVE multiply.
  - O-projection contracts attn^T [d_local, s] tiles against w_o columns;
    per-core partial outputs are summed (+b_o) on the host.

All matmuls run in float32r (full PE rate at N=512, ~1.6e-4 rel err).
"""
import numpy as np

import concourse.bass as bass
import concourse.mybir as mybir
from concourse.tile import TileContext
from concourse.bass_utils import run_bass_kernel_spmd

F32 = mybir.dt.float32
F32R = mybir.dt.float32r
AF = mybir.ActivationFunctionType

B, S, D = 4, 2048, 1024
H, DH = 16, 64
HL = 8        # heads per core
DL = HL * DH  # 512 local model dims
P = 128
SC = 512      # s-chunk width
NCH = S // SC  # 4 s-chunks
ND = D // P    # 8 contraction subtiles for D
NM = DL // P   # 4 m-tiles of local outputs
NT = S // P    # 16 t-tiles

_MAX_WAITS = 1


def _split_excess_waits(nc, max_waits=_MAX_WAITS):
    """walrus here rejects >1 sync-wait per instruction; spill extras onto
    same-engine NoOps inserted before the instruction."""
    f = nc.m.functions[0]
    n = 0
    for bb in f.blocks:
        changed = False
        out = []
        for inst in bb.instructions:
            si = inst.sync_info
            if si is not None and len(si.on_wait) > max_waits:
                waits = list(si.on_wait)
                keep = waits[-max_waits:]
                spill = waits[:-max_waits]
                for i in range(0, len(spill), max_waits):
                    nop = mybir.InstNoOp(name=f"WSPILL-{n}", ins=[], outs=[])
                    n += 1
                    nop.engine = inst.engine
                    nop.sync_info = mybir.SyncInfo(
                        on_wait=spill[i : i + max_waits], on_update=[]
                    )
                    nc.register_instruction(nop, overwrite=True)
                    out.append(nop)
                inst.sync_info = mybir.SyncInfo(
                    on_wait=keep, on_update=list(si.on_update)
                )
                changed = True
            out.append(inst)
        if changed:
            bb.instructions = out
    return n


def build():
    nc = bass.Bass()
    xq = nc.dram_tensor("xq", [D, S], F32R, kind="ExternalInput")
    xk = nc.dram_tensor("xk", [D, S], F32R, kind="ExternalInput")
    xv = nc.dram_tensor("xv", [D, S], F32R, kind="ExternalInput")
    wq = nc.dram_tensor("wq", [D, DL], F32R, kind="ExternalInput")
    wk = nc.dram_tensor("wk", [D, DL], F32R, kind="ExternalInput")
    wv = nc.dram_tensor("wv", [D, DL], F32R, kind="ExternalInput")
    wo = nc.dram_tensor("wo", [DL, D], F32R, kind="ExternalInput")
    bq = nc.dram_tensor("bq", [DL], F32, kind="ExternalInput")
    bk = nc.dram_tensor("bk", [DL], F32, kind="ExternalInput")
    bv = nc.dram_tensor("bv", [DL], F32R, kind="ExternalInput")
    out = nc.dram_tensor("out", [S, D], F32, kind="ExternalOutput")

    xq_r = xq.rearrange("(o p) s -> p o s", p=P)
    xk_r = xk.rearrange("(o p) s -> p o s", p=P)
    xv_r = xv.rearrange("(o p) s -> p o s", p=P)
    wq_r = wq.rearrange("(o p) m -> p o m", p=P)
    wk_r = wk.rearrange("(o p) m -> p o m", p=P)
    wv_r = wv.rearrange("(o p) m -> p o m", p=P)
    wo_r = wo.rearrange("(o p) n -> p o n", p=P)
    out_r = out.rearrange("(so p) n -> p so n", p=P)

    with TileContext(nc) as tc:
        with (
            tc.tile_pool(name="persist", bufs=1) as persist,
            tc.tile_pool(name="wpool", bufs=1) as wpool,
            tc.tile_pool(name="xpool", bufs=3) as xpool,
            tc.tile_pool(name="expp", bufs=2) as expp,
            tc.tile_pool(name="attnp", bufs=2) as attnp,
            tc.tile_pool(name="osb", bufs=2) as osbp,
            tc.tile_pool(name="nrm", bufs=2) as nrm,
            tc.tile_pool(name="ps_big", bufs=1, space="PSUM") as ps_big,
            tc.tile_pool(name="ps_pv", bufs=1, space="PSUM") as ps_pv,
            tc.tile_pool(name="ps_sm", bufs=2, space="PSUM") as ps_sm,
        ):
            qt = persist.tile([P, NM, S], F32R, tag="qt")
            kt = persist.tile([P, NM, S], F32R, tag="kt")
            vaug = persist.tile([P, NT, HL, 66], F32R, tag="vaug")
            wq_t = persist.tile([P, ND, DL], F32R, tag="wq")
            ones_f = persist.tile([P, P], F32, tag="ones_f")
            ones_r = persist.tile([P, P], F32R, tag="ones_r")
            bq_sb = persist.tile([P, NM], F32, tag="bq")
            bk_sb = persist.tile([P, NM], F32, tag="bk")
            bv_t = persist.tile([P, DL], F32R, tag="bv_t")
            bv_bc = persist.tile([P, DL], F32, tag="bv_bc")

            # ---- constants / biases ----
            nc.vector.memset(ones_f[:], 1.0)
            nc.vector.tensor_copy(ones_r[:], ones_f[:])
            nc.sync.dma_start(bq_sb[:], bq.rearrange("(o p) -> p o", p=P))
            nc.sync.dma_start(bk_sb[:], bk.rearrange("(o p) -> p o", p=P))
            nc.sync.dma_start(bv_t[0:1, :], bv[None, :])
            ps = ps_sm.tile([P, SC], F32, tag="sm")
            nc.tensor.matmul(ps[:], ones_r[0:1, 0:P], bv_t[0:1, :], start=True,
                             stop=True)
            nc.vector.tensor_copy(bv_bc[:], ps[:])
            # ones columns of V_aug
            of = ones_f[:, 0:NT * HL].rearrange("p (a b) -> p a b", a=NT)
            nc.vector.tensor_copy(vaug[:, :, :, 0:1], of[:, :, :, None])
            nc.vector.tensor_copy(vaug[:, :, :, 65:66], of[:, :, :, None])

            def proj_chunk(dst, w_tile, x_r, c, bias_sb):
                xa = xpool.tile([P, ND // 2, SC], F32R, tag="x")
                xb = xpool.tile([P, ND // 2, SC], F32R, tag="x")
                nc.sync.dma_start(xa[:], x_r[:, 0 : ND // 2, c * SC : (c + 1) * SC])
                nc.sync.dma_start(xb[:], x_r[:, ND // 2 : ND, c * SC : (c + 1) * SC])
                halves = (xa, xb)
                for m in range(NM):
                    psm = ps_sm.tile([P, SC], F32, tag="sm")
                    for k in range(ND):
                        nc.tensor.matmul(
                            psm[:],
                            w_tile[:, k, m * P : (m + 1) * P],
                            halves[k // 4][:, k % 4, :],
                            start=(k == 0),
                            stop=(k == ND - 1),
                        )
                    nc.vector.tensor_add(
                        dst[:, m, c * SC : (c + 1) * SC],
                        psm[:],
                        bias_sb[:, m : m + 1].to_broadcast((P, SC)),
                    )

            def vproj_chunk(wv_t, c):
                xa = xpool.tile([P, ND // 2, SC], F32R, tag="x")
                xb = xpool.tile([P, ND // 2, SC], F32R, tag="x")
                nc.sync.dma_start(xa[:], xv_r[:, 0 : ND // 2, c * SC : (c + 1) * SC])
                nc.sync.dma_start(xb[:], xv_r[:, ND // 2 : ND, c * SC : (c + 1) * SC])
                halves = (xa, xb)
                for i in range(4):
                    t_o = c * 4 + i
                    psm = ps_sm.tile([P, SC], F32, tag="sm")
                    for k in range(ND):
                        nc.tensor.matmul(
                            psm[:],
                            halves[k // 4][:, k % 4, i * P : (i + 1) * P],
                            wv_t[:, k, :],
                            start=(k == 0),
                            stop=(k == ND - 1),
                        )
                    for h in range(HL):
                        nc.vector.tensor_add(
                            vaug[:, t_o, h, 1:65],
                            psm[:, h * DH : (h + 1) * DH],
                            bv_bc[:, h * DH : (h + 1) * DH],
                        )

            def normalize_pre(pv):
                """Consume the PV psum right away on DVE (frees the psum slot):
                copy unnormalized rows, reciprocal of the denominator row."""
                raw = nrm.tile([P, SC], F32, tag="raw")
                rec = nrm.tile([P, SC], F32R, tag="rec")
                nc.vector.tensor_copy(raw[0:64, :], pv[0:64, :])
                nc.vector.tensor_copy(raw[64:65, :], pv[64:65, :])
                with nc.allow_low_precision(reason="fp32r recip for matmul rhs"):
                    nc.vector.reciprocal(rec[64:65, :], raw[64:65, :])
                return raw, rec

            def normalize_post(raw, rec, dst_lo, dst_hi_dma):
                """Broadcast 1/denom across partitions (K=1 matmul) and apply."""
                bc = ps_sm.tile([P, SC], F32, tag="sm")
                nc.tensor.matmul(bc[0:64, :], ones_r[64:65, 0:64], rec[64:65, :],
                                 start=True, stop=True)
                if dst_hi_dma is None:
                    nc.vector.tensor_mul(dst_lo, bc[0:64, :], raw[0:64, :])
                else:
                    tmp = nrm.tile([P, SC], F32R, tag="tmp")
                    nc.vector.tensor_mul(tmp[0:64, :], bc[0:64, :], raw[0:64, :])
                    nc.sync.dma_start(dst_hi_dma, tmp[0:64, :])

            # ---- projections (prefix) ----
            nc.sync.dma_start(wq_t[:], wq_r[:])
            proj_chunk(qt, wq_t, xq_r, 0, bq_sb)
            wk_t = wpool.tile([P, ND, DL], F32R, tag="w")
            nc.sync.dma_start(wk_t[:], wk_r[:])
            for c in range(NCH):
                proj_chunk(kt, wk_t, xk_r, c, bk_sb)
            wv_t = wpool.tile([P, ND, DL], F32R, tag="w")
            nc.sync.dma_start(wv_t[:], wv_r[:])
            for c in range(NCH):
                vproj_chunk(wv_t, c)
            wo_t = wpool.tile([P, NM, D], F32R, tag="w")
            nc.sync.dma_start(wo_t[:], wo_r[:])

            # ---- attention + o-proj, per s-chunk ----
            def oproj_chunk(c, attn_t):
                for st in range(4):
                    for n in range(2):
                        psm = ps_sm.tile([P, SC], F32, tag="sm")
                        for do in range(NM):
                            nc.tensor.matmul(
                                psm[:],
                                attn_t[:, do, st * P : (st + 1) * P],
                                wo_t[:, do, n * SC : (n + 1) * SC],
                                start=(do == 0),
                                stop=(do == NM - 1),
                            )
                        ob = osbp.tile([P, SC], F32, tag="ob")
                        nc.vector.tensor_copy(ob[:], psm[:])
                        nc.sync.dma_start(
                            out_r[:, c * 4 + st, n * SC : (n + 1) * SC], ob[:]
                        )

            pending_oproj = None
            for c in range(NCH):
                attn_t = attnp.tile([P, NM, SC], F32R, tag="attn")
                cs = slice(c * SC, (c + 1) * SC)
                deferred = None
                for p in range(NM):
                    pv_e = ps_pv.tile([P, SC], F32, tag="pv_e")
                    pv_o = ps_pv.tile([P, SC], F32, tag="pv_o")

                    def emit_pv(ex, ta, tb, p=p, pv_e=pv_e, pv_o=pv_o):
                        for j, (t, hh) in enumerate(
                            [(ta, 2 * p), (tb, 2 * p), (ta, 2 * p + 1),
                             (tb, 2 * p + 1)]
                        ):
                            pv = pv_e if j < 2 else pv_o
                            nc.tensor.matmul(
                                pv[0:65, :],
                                vaug[:, t, hh, 1:66],
                                ex[:, j, :],
                                start=(t == 0),
                                stop=(t == NT - 1),
                            )

                    pend = None
                    for g in range(NT // 2):
                        ta, tb = 2 * g, 2 * g + 1
                        big = ps_big.tile([P, 4, SC], F32, tag="big")
                        for j, (t, lo) in enumerate(
                            [(ta, 0), (tb, 0), (ta, 64), (tb, 64)]
                        ):
                            nc.tensor.matmul(
                                big[:, j, :],
                                kt[lo : lo + 64, p, t * P : (t + 1) * P],
                                qt[lo : lo + 64, p, cs],
                                start=True,
                                stop=True,
                                tile_position=(lo, 0),
                            )
                        ex = expp.tile([P, 4, SC], F32R, tag="ex")
                        nc.scalar.activation(ex[:], big[:], AF.Exp, scale=0.125)
                        if g == 4 and deferred is not None:
                            deferred()
                            deferred = None
                        if g == 5 and p == 0 and pending_oproj is not None:
                            pending_oproj()
                            pending_oproj = None
                        if g == 2 and p == 3 and c < NCH - 1:
                            proj_chunk(qt, wq_t, xq_r, c + 1, bq_sb)
                        if pend is not None:
                            emit_pv(*pend)
                        pend = (ex, ta, tb)
                    emit_pv(*pend)
                    raw_e, rec_e = normalize_pre(pv_e)
                    raw_o, rec_o = normalize_pre(pv_o)

                    def make_deferred(raw_e=raw_e, rec_e=rec_e, raw_o=raw_o,
                                      rec_o=rec_o, p=p, attn_t=attn_t):
                        def run():
                            normalize_post(raw_e, rec_e, attn_t[0:64, p, :], None)
                            normalize_post(raw_o, rec_o, None,
                                           attn_t[64:128, p, :])
                        return run

                    deferred = make_deferred()
                deferred()

                def make_oproj(c=c, attn_t=attn_t):
                    return lambda: oproj_chunk(c, attn_t)

                pending_oproj = make_oproj()
            pending_oproj()

    _split_excess_waits(nc)
    return nc


_CACHE = {}


def _get_nc():
    if "nc" not in _CACHE:
        _CACHE["nc"] = build()
    return _CACHE["nc"]


def _f32(x):
    return np.asarray(x).astype(np.float32, copy=False)


def _prep_core_inputs(c, q, k, v, w_q, b_q, w_k, b_k, w_v, b_v, w_o, b_o):
    b, hg = c // 2, c % 2
    hs = slice(hg * DL, hg * DL + DL)
    return {
        "xq": np.ascontiguousarray(q[b].T),
        "xk": np.ascontiguousarray(k[b].T),
        "xv": np.ascontiguousarray(v[b].T),
        "wq": np.ascontiguousarray(w_q[hs, :].T),
        "wk": np.ascontiguousarray(w_k[hs, :].T),
        "wv": np.ascontiguousarray(w_v[hs, :].T),
        "wo": np.ascontiguousarray(w_o[:, hs].T),
        "bq": np.ascontiguousarray(b_q[hs]),
        "bk": np.ascontiguousarray(b_k[hs]),
        "bv": np.ascontiguousarray(b_v[hs]),
    }


def kernel(q, k, v, w_q, b_q, w_k, b_k, w_v, b_v, w_o, b_o):
    q, k, v = _f32(q), _f32(k), _f32(v)
    w_q, b_q = _f32(w_q), _f32(b_q)
    w_k, b_k = _f32(w_k), _f32(b_k)
    w_v, b_v = _f32(w_v), _f32(b_v)
    w_o, b_o = _f32(w_o), _f32(b_o)

    nc = _get_nc()
    in_maps = [
        _prep_core_inputs(c, q, k, v, w_q, b_q, w_k, b_k, w_v, b_v, w_o, b_o)
        for c in range(8)
    ]
    res = run_bass_kernel_spmd(nc, in_maps, core_ids=list(range(8)))
    out = np.empty((B, S, D), np.float32)
    for b in range(B):
        out[b] = res.results[2 * b]["out"] + res.results[2 * b + 1]["out"] + b_o
    return out
